# revision 24
# baseline (speedup 1.0000x reference)
"""Trainium2 Bass kernel for the AR(t) recurrence problem.

Math: the recurrence  x_i = sum_j params[j] * x_{i-1-j} + bias  (i in [t, 2t))
is affine in the seed window, so the whole output is

    out = inputs @ M + c

where M [t, t] and c [t] depend only on params/bias. M's columns obey
    m_{k+1} = shift_down(m_k) + m_k[t-1] * p_rev,   m_0 = p_rev
(p_rev = params reversed), an O(t^2) host-side precompute. The device then
does a single dense matmul, data-parallel over 8 cores (512 rows each):

    per core: out_shard[512, 2048] = inT.T @ M + c
    - inT (input shard, transposed on host, bf16) streams as the stationary
      operand in 128x128 tiles
    - M (bf16) streams as the moving operand in [128, 512] tiles
    - accumulate over the 16 contraction tiles into PSUM [128, 2048] strips
    - add c (fp32, exact) during PSUM->SBUF evacuation, DMA out fp32

bf16 is ample precision here: the data-dependent term is strongly
attenuated by the filter (|M| columns ~1e-2 norm) while c is added in fp32;
measured max elementwise relative error vs the fp32 reference ~1.5e-4.
"""

import numpy as np
import ml_dtypes

B = 4096          # batch rows
T = 2048          # time steps == contraction length
NCORES = 8
BS = B // NCORES  # 512 rows per core
P = 128           # partitions
NJ = T // P       # 16 contraction tiles
KB = 512          # matmul moving free-dim (one PSUM bank of fp32)
NKB = T // KB     # 4 k-blocks
NBT = BS // P     # 4 output row-tiles per core

_cache = {}


def _build_and_compile():
    import concourse.mybir as mybir
    from concourse import bacc
    from concourse.tile import TileContext

    nc = bacc.Bacc(
        "TRN2",
        target_bir_lowering=False,
        debug=False,
        enable_asserts=False,
        num_devices=NCORES,
    )
    inT = nc.dram_tensor("art_inT", [T, BS], mybir.dt.bfloat16, kind="ExternalInput")
    mmat = nc.dram_tensor("art_m", [T, T], mybir.dt.bfloat16, kind="ExternalInput")
    cvec = nc.dram_tensor("art_c", [P, T], mybir.dt.float32, kind="ExternalInput")
    out = nc.dram_tensor("art_out", [BS, T], mybir.dt.float32, kind="ExternalOutput")

    with TileContext(nc) as tc:
        with (
            tc.tile_pool(name="weights", bufs=NJ) as mpool,
            tc.tile_pool(name="acts", bufs=NJ) as ipool,
            tc.tile_pool(name="consts", bufs=1) as cpool,
            tc.tile_pool(name="outs", bufs=2) as opool,
            tc.tile_pool(name="warm", bufs=1) as wpool,
            tc.tile_pool(name="acc", bufs=2, space="PSUM") as pspool,
        ):
            # Scratch operand for the PE warm-up matmuls (HAM clock gate).
            wt = wpool.tile([P, KB], mybir.dt.bfloat16, name="wt")
            nc.vector.memset(wt[:], 0.0)

            c_sb = cpool.tile([P, T], mybir.dt.float32, name="c_sb")

            # Loads alternate between the two HWDGE rings (sync + scalar) so
            # descriptor-generation (~600ns per dma_start) is not the pacing
            # bottleneck; m[n] and in[n] ride opposite rings so the first m
            # tile is not queued behind the first in tile.
            in_tiles, m_tiles = [], []
            for n in range(NJ):
                mq = nc.scalar if n % 2 == 0 else nc.sync
                iq = nc.sync if n % 2 == 0 else nc.scalar
                it = ipool.tile([P, BS], mybir.dt.bfloat16, tag="in", name=f"in_sb{n}")
                iq.dma_start(out=it[:], in_=inT[n * P : (n + 1) * P, :])
                mt = mpool.tile([P, T], mybir.dt.bfloat16, tag="m", name=f"m_sb{n}")
                mq.dma_start(out=mt[:], in_=mmat[n * P : (n + 1) * P, :])
                in_tiles.append(it)
                m_tiles.append(mt)
            # c is first needed by the pass-A evacuation (~43us in); issuing
            # it after the m/in streams keeps it from delaying any m tile.
            nc.sync.dma_start(out=c_sb[:], in_=cvec[:])

            # Two passes of two row-tiles each: 2 PSUM strips of [128, 2048]
            # (4 banks each) per pass fills all 8 banks; the n-loop touches
            # each streamed m tile once per pass so the first pass overlaps
            # the M DMA.
            for half in range(NBT // 2):
                ps = [
                    pspool.tile([P, T], mybir.dt.float32, tag="ps", name=f"ps{half}_{bi}")
                    for bi in range(2)
                ]
                if half == 0:
                    # Dummy matmuls on memset data while the first DMAs
                    # land, so the HAM clock gate opens to 8/8 before the
                    # real matmuls. Sized to end just past m0's arrival
                    # (~10.8us) -- more would delay the real stream. Each is
                    # its own start/stop group; the real n==0 matmul
                    # re-clears the bank.
                    for i in range(5):
                        nc.tensor.matmul(ps[0][:, :KB], wt[:, :P], wt[:])
                def mm(bi, n):
                    bt = half * 2 + bi
                    lhsT = in_tiles[n][:, bt * P : (bt + 1) * P]
                    for kb in range(NKB):
                        nc.tensor.matmul(
                            ps[bi][:, kb * KB : (kb + 1) * KB],
                            lhsT,
                            m_tiles[n][:, kb * KB : (kb + 1) * KB],
                            start=(n == 0),
                            stop=(n == NJ - 1),
                        )

                # Interleave b-tiles over most of the contraction (matches
                # the M DMA arrival order), but run the last `split`
                # iterations per-b-tile so strip 0's accumulation closes
                # early: its serial DVE evacuation then hides under strip
                # 1's remaining matmuls instead of stalling what follows
                # (pass transition, kernel tail). Pass A keeps split small
                # (m14/m15 DMA arrival is tight); pass B is SBUF-resident.
                split = 4 if half == 0 else 6
                for n in range(NJ - split):
                    for bi in range(2):
                        mm(bi, n)
                for bi in range(2):
                    if half == 1 and bi == 1:
                        # Very last strip: kb-major so each PSUM bank's
                        # accumulation group closes progressively early and
                        # the DVE streams through all 8 tail adds without
                        # idling; only bank 3's add+store trail the last
                        # matmul.
                        bt = half * 2 + bi
                        for kb in range(NKB):
                            for n in range(NJ - split, NJ):
                                nc.tensor.matmul(
                                    ps[bi][:, kb * KB : (kb + 1) * KB],
                                    in_tiles[n][:, bt * P : (bt + 1) * P],
                                    m_tiles[n][:, kb * KB : (kb + 1) * KB],
                                    start=False,
                                    stop=(n == NJ - 1),
                                )
                    else:
                        for n in range(NJ - split, NJ):
                            mm(bi, n)
                for bi in range(2):
                    bt = half * 2 + bi
                    stq = nc.sync if bi == 0 else nc.scalar
                    ot = opool.tile([P, T], mybir.dt.float32, tag="o", name=f"o_sb{bt}")
                    for kb in range(NKB):
                        ksl = slice(kb * KB, (kb + 1) * KB)
                        nc.vector.tensor_add(
                            out=ot[:, ksl], in0=ps[bi][:, ksl], in1=c_sb[:, ksl]
                        )
                        stq.dma_start(
                            out=out[bt * P : (bt + 1) * P, ksl], in_=ot[:, ksl]
                        )

    nc.compile()
    return nc


def _build_M_c(params, bias):
    """M [t, t], c [t] (float64) such that out = inputs @ M + c."""
    t = params.shape[0]
    p_rev = params[::-1].astype(np.float64)
    M = np.empty((t, t), np.float64)
    col = p_rev.copy()
    M[:, 0] = col
    for k in range(1, t):
        last = col[-1]
        shifted = np.empty_like(col)
        shifted[0] = 0.0
        shifted[1:] = col[:-1]
        col = shifted + last * p_rev
        M[:, k] = col
    b = np.float64(bias[0])
    u = np.zeros(t, np.float64)
    c = np.empty(t, np.float64)
    for k in range(t):
        nv = u @ p_rev + b
        c[k] = nv
        u = np.roll(u, -1)
        u[-1] = nv
    return M, c


def _make_in_maps(inputs, params, bias):
    M, c = _build_M_c(params, bias)
    m_bf = M.astype(np.float32).astype(ml_dtypes.bfloat16)
    c128 = np.ascontiguousarray(
        np.broadcast_to(c.astype(np.float32)[None, :], (P, T))
    )
    in_bf = inputs.astype(ml_dtypes.bfloat16)
    in_maps = []
    for s in range(NCORES):
        shard = np.ascontiguousarray(in_bf[s * BS : (s + 1) * BS, :].T)
        in_maps.append({"art_inT": shard, "art_m": m_bf, "art_c": c128})
    return in_maps


def run(inputs, params, bias, **spmd_kwargs):
    """Build in_maps, run the SPMD kernel, return (output, BassKernelResults)."""
    from concourse.bass_utils import run_bass_kernel_spmd

    if "nc" not in _cache:
        _cache["nc"] = _build_and_compile()
    nc = _cache["nc"]

    inputs = np.ascontiguousarray(np.asarray(inputs, dtype=np.float32))
    params = np.asarray(params, dtype=np.float32)
    bias = np.asarray(bias, dtype=np.float32)
    assert inputs.shape == (B, T), inputs.shape
    assert params.shape == (T,), params.shape
    in_maps = _make_in_maps(inputs, params, bias)
    res = run_bass_kernel_spmd(nc, in_maps, core_ids=list(range(NCORES)), **spmd_kwargs)
    out = np.concatenate([r["art_out"] for r in res.results], axis=0)
    return out, res


def kernel(inputs, params, bias):
    out, _ = run(inputs, params, bias)
    return out


# revision 25
# speedup vs baseline: 1.1323x; 1.1323x over previous
"""Trainium2 Bass kernel for the AR(t) recurrence problem.

Math: the recurrence  x_i = sum_j params[j] * x_{i-1-j} + bias  (i in [t, 2t))
is affine in the seed window, so the whole output is

    out = inputs @ M + c

where M [t, t] and c [t] depend only on params/bias. M's columns obey
    m_{k+1} = shift_down(m_k) + m_k[t-1] * p_rev,   m_0 = p_rev
(p_rev = params reversed), an O(t^2) host-side precompute. The device then
does a single dense matmul, data-parallel over 8 cores (512 rows each):

    per core: out_shard[512, 2048] = inT.T @ M + c
    - inT (input shard, transposed on host, bf16) streams as the stationary
      operand in 128x128 tiles
    - M (bf16) streams as the moving operand in [128, 512] tiles
    - accumulate over the 16 contraction tiles into PSUM [128, 2048] strips
    - add c (fp32, exact) during PSUM->SBUF evacuation, DMA out fp32

bf16 is ample precision here: the data-dependent term is strongly
attenuated by the filter (|M| columns ~1e-2 norm) while c is added in fp32;
measured max elementwise relative error vs the fp32 reference ~1.5e-4.
"""

import numpy as np
import ml_dtypes

B = 4096          # batch rows
T = 2048          # time steps == contraction length
NCORES = 8
BS = B // NCORES  # 512 rows per core
P = 128           # partitions
NJ = T // P       # 16 contraction tiles
KB = 512          # matmul moving free-dim (one PSUM bank of fp32)
NKB = T // KB     # 4 k-blocks
NBT = BS // P     # 4 output row-tiles per core

_cache = {}


def _build_and_compile():
    import concourse.mybir as mybir
    from concourse import bacc
    from concourse.tile import TileContext

    nc = bacc.Bacc(
        "TRN2",
        target_bir_lowering=False,
        debug=False,
        enable_asserts=False,
        num_devices=NCORES,
    )
    inT = nc.dram_tensor("art_inT", [T, BS], mybir.dt.bfloat16, kind="ExternalInput")
    mmat = nc.dram_tensor("art_m", [T, T], mybir.dt.bfloat16, kind="ExternalInput")
    cvec = nc.dram_tensor("art_c", [P, T], mybir.dt.float32, kind="ExternalInput")
    out = nc.dram_tensor("art_out", [BS, T], mybir.dt.float32, kind="ExternalOutput")

    with TileContext(nc) as tc:
        with (
            tc.tile_pool(name="weights", bufs=NJ) as mpool,
            tc.tile_pool(name="acts", bufs=NJ) as ipool,
            tc.tile_pool(name="consts", bufs=1) as cpool,
            tc.tile_pool(name="outs", bufs=2) as opool,
            tc.tile_pool(name="warm", bufs=1) as wpool,
            tc.tile_pool(name="acc", bufs=8, space="PSUM") as pspool,
        ):
            # Scratch operand for the PE warm-up matmuls (HAM clock gate).
            wt = wpool.tile([P, KB], mybir.dt.bfloat16, name="wt")
            nc.vector.memset(wt[:], 0.0)

            c_sb = cpool.tile([P, T], mybir.dt.float32, name="c_sb")

            # Loads alternate between the two HWDGE rings (sync + scalar) so
            # descriptor-generation (~600ns per dma_start) is not the pacing
            # bottleneck; m[n] and in[n] ride opposite rings so the first m
            # tile is not queued behind the first in tile.
            in_tiles, m_tiles = [], []
            for n in range(NJ):
                mq = nc.scalar if n % 2 == 0 else nc.sync
                iq = nc.sync if n % 2 == 0 else nc.scalar
                it = ipool.tile([P, BS], mybir.dt.bfloat16, tag="in", name=f"in_sb{n}")
                iq.dma_start(out=it[:], in_=inT[n * P : (n + 1) * P, :])
                mt = mpool.tile([P, T], mybir.dt.bfloat16, tag="m", name=f"m_sb{n}")
                mq.dma_start(out=mt[:], in_=mmat[n * P : (n + 1) * P, :])
                in_tiles.append(it)
                m_tiles.append(mt)
            # c is first needed by the pass-A evacuation (~43us in); issuing
            # it after the m/in streams keeps it from delaying any m tile.
            nc.sync.dma_start(out=c_sb[:], in_=cvec[:])

            # Two passes of two row-tiles each: 2 PSUM strips of [128, 2048]
            # (4 banks each) per pass fills all 8 banks; the n-loop touches
            # each streamed m tile once per pass so the first pass overlaps
            # the M DMA.
            for half in range(NBT // 2):
                # One 1-bank PSUM tile per (row-tile, k-block) unit: Tile
                # serializes DVE reads vs PE writes at tile granularity, so
                # separate tiles let each bank's evacuation start as soon as
                # its own accumulation group closes.
                ps = [
                    [
                        pspool.tile(
                            [P, KB], mybir.dt.float32, tag="ps", name=f"ps{half}_{bi}_{kb}"
                        )
                        for kb in range(NKB)
                    ]
                    for bi in range(2)
                ]
                if half == 0:
                    # Dummy matmuls on memset data while the first DMAs
                    # land, so the HAM clock gate opens to 8/8 before the
                    # real matmuls. Sized to end just past m0's arrival
                    # (~10.8us) -- more would delay the real stream. Each is
                    # its own start/stop group; the real n==0 matmul
                    # re-clears the bank.
                    for i in range(5):
                        nc.tensor.matmul(ps[0][0][:], wt[:, :P], wt[:])
                def mm(bi, n):
                    bt = half * 2 + bi
                    lhsT = in_tiles[n][:, bt * P : (bt + 1) * P]
                    for kb in range(NKB):
                        nc.tensor.matmul(
                            ps[bi][kb][:],
                            lhsT,
                            m_tiles[n][:, kb * KB : (kb + 1) * KB],
                            start=(n == 0),
                            stop=(n == NJ - 1),
                        )

                # Interleave b-tiles over most of the contraction (matches
                # the M DMA arrival order), but run the last `split`
                # iterations per-b-tile so strip 0's accumulation closes
                # early: its serial DVE evacuation then hides under strip
                # 1's remaining matmuls instead of stalling what follows
                # (pass transition, kernel tail). Pass A keeps split small
                # (m14/m15 DMA arrival is tight); pass B is SBUF-resident.
                split = 4 if half == 0 else 6
                for n in range(NJ - split):
                    for bi in range(2):
                        mm(bi, n)
                for bi in range(2):
                    if half == 1 and bi == 1:
                        # Very last strip: kb-major so each PSUM bank's
                        # accumulation group closes progressively early and
                        # the DVE streams through all 8 tail adds without
                        # idling; only bank 3's add+store trail the last
                        # matmul.
                        bt = half * 2 + bi
                        for kb in range(NKB):
                            for n in range(NJ - split, NJ):
                                nc.tensor.matmul(
                                    ps[bi][kb][:],
                                    in_tiles[n][:, bt * P : (bt + 1) * P],
                                    m_tiles[n][:, kb * KB : (kb + 1) * KB],
                                    start=False,
                                    stop=(n == NJ - 1),
                                )
                    else:
                        for n in range(NJ - split, NJ):
                            mm(bi, n)
                for bi in range(2):
                    bt = half * 2 + bi
                    stq = nc.sync if bi == 0 else nc.scalar
                    ot = opool.tile([P, T], mybir.dt.float32, tag="o", name=f"o_sb{bt}")
                    for kb in range(NKB):
                        ksl = slice(kb * KB, (kb + 1) * KB)
                        nc.vector.tensor_add(
                            out=ot[:, ksl], in0=ps[bi][kb][:], in1=c_sb[:, ksl]
                        )
                        stq.dma_start(
                            out=out[bt * P : (bt + 1) * P, ksl], in_=ot[:, ksl]
                        )

    nc.compile()
    return nc


def _build_M_c(params, bias):
    """M [t, t], c [t] (float64) such that out = inputs @ M + c."""
    t = params.shape[0]
    p_rev = params[::-1].astype(np.float64)
    M = np.empty((t, t), np.float64)
    col = p_rev.copy()
    M[:, 0] = col
    for k in range(1, t):
        last = col[-1]
        shifted = np.empty_like(col)
        shifted[0] = 0.0
        shifted[1:] = col[:-1]
        col = shifted + last * p_rev
        M[:, k] = col
    b = np.float64(bias[0])
    u = np.zeros(t, np.float64)
    c = np.empty(t, np.float64)
    for k in range(t):
        nv = u @ p_rev + b
        c[k] = nv
        u = np.roll(u, -1)
        u[-1] = nv
    return M, c


def _make_in_maps(inputs, params, bias):
    M, c = _build_M_c(params, bias)
    m_bf = M.astype(np.float32).astype(ml_dtypes.bfloat16)
    c128 = np.ascontiguousarray(
        np.broadcast_to(c.astype(np.float32)[None, :], (P, T))
    )
    in_bf = inputs.astype(ml_dtypes.bfloat16)
    in_maps = []
    for s in range(NCORES):
        shard = np.ascontiguousarray(in_bf[s * BS : (s + 1) * BS, :].T)
        in_maps.append({"art_inT": shard, "art_m": m_bf, "art_c": c128})
    return in_maps


def run(inputs, params, bias, **spmd_kwargs):
    """Build in_maps, run the SPMD kernel, return (output, BassKernelResults)."""
    from concourse.bass_utils import run_bass_kernel_spmd

    if "nc" not in _cache:
        _cache["nc"] = _build_and_compile()
    nc = _cache["nc"]

    inputs = np.ascontiguousarray(np.asarray(inputs, dtype=np.float32))
    params = np.asarray(params, dtype=np.float32)
    bias = np.asarray(bias, dtype=np.float32)
    assert inputs.shape == (B, T), inputs.shape
    assert params.shape == (T,), params.shape
    in_maps = _make_in_maps(inputs, params, bias)
    res = run_bass_kernel_spmd(nc, in_maps, core_ids=list(range(NCORES)), **spmd_kwargs)
    out = np.concatenate([r["art_out"] for r in res.results], axis=0)
    return out, res


def kernel(inputs, params, bias):
    out, _ = run(inputs, params, bias)
    return out


# revision 27
# speedup vs baseline: 1.1499x; 1.0155x over previous
"""Trainium2 Bass kernel for the AR(t) recurrence problem.

Math: the recurrence  x_i = sum_j params[j] * x_{i-1-j} + bias  (i in [t, 2t))
is affine in the seed window, so the whole output is

    out = inputs @ M + c

where M [t, t] and c [t] depend only on params/bias. M's columns obey
    m_{k+1} = shift_down(m_k) + m_k[t-1] * p_rev,   m_0 = p_rev
(p_rev = params reversed), an O(t^2) host-side precompute. The device then
does a single dense matmul, data-parallel over 8 cores (512 rows each):

    per core: out_shard[512, 2048] = inT.T @ M + c
    - inT (input shard, transposed on host, bf16) streams as the stationary
      operand in 128x128 tiles
    - M (bf16) streams as the moving operand in [128, 512] tiles
    - accumulate over the 16 contraction tiles into PSUM [128, 2048] strips
    - add c (fp32, exact) during PSUM->SBUF evacuation, DMA out fp32

bf16 is ample precision here: the data-dependent term is strongly
attenuated by the filter (|M| columns ~1e-2 norm) while c is added in fp32;
measured max elementwise relative error vs the fp32 reference ~1.5e-4.
"""

import numpy as np
import ml_dtypes

B = 4096          # batch rows
T = 2048          # time steps == contraction length
NCORES = 8
BS = B // NCORES  # 512 rows per core
P = 128           # partitions
NJ = T // P       # 16 contraction tiles
KB = 512          # matmul moving free-dim (one PSUM bank of fp32)
NKB = T // KB     # 4 k-blocks
NBT = BS // P     # 4 output row-tiles per core

_cache = {}


def _build_and_compile():
    import concourse.mybir as mybir
    from concourse import bacc
    from concourse.tile import TileContext

    nc = bacc.Bacc(
        "TRN2",
        target_bir_lowering=False,
        debug=False,
        enable_asserts=False,
        num_devices=NCORES,
    )
    inT = nc.dram_tensor("art_inT", [T, BS], mybir.dt.bfloat16, kind="ExternalInput")
    mmat = nc.dram_tensor("art_m", [T, T], mybir.dt.bfloat16, kind="ExternalInput")
    cvec = nc.dram_tensor("art_c", [P, T], mybir.dt.float32, kind="ExternalInput")
    out = nc.dram_tensor("art_out", [BS, T], mybir.dt.float32, kind="ExternalOutput")

    with TileContext(nc) as tc:
        with (
            tc.tile_pool(name="weights", bufs=NJ) as mpool,
            tc.tile_pool(name="acts", bufs=NJ) as ipool,
            tc.tile_pool(name="consts", bufs=1) as cpool,
            tc.tile_pool(name="outs", bufs=2) as opool,
            tc.tile_pool(name="warm", bufs=1) as wpool,
            tc.tile_pool(name="acc", bufs=8, space="PSUM") as pspool,
        ):
            # Scratch operand for the PE warm-up matmuls (HAM clock gate).
            wt = wpool.tile([P, KB], mybir.dt.bfloat16, name="wt")
            nc.vector.memset(wt[:], 0.0)

            c_sb = cpool.tile([P, T], mybir.dt.float32, name="c_sb")

            # Loads alternate between the two HWDGE rings (sync + scalar) so
            # descriptor-generation (~600ns per dma_start) is not the pacing
            # bottleneck; m[n] and in[n] ride opposite rings so the first m
            # tile is not queued behind the first in tile.
            in_tiles, m_tiles = [], []
            for n in range(NJ):
                mq = nc.scalar if n % 2 == 0 else nc.sync
                iq = nc.sync if n % 2 == 0 else nc.scalar
                it = ipool.tile([P, BS], mybir.dt.bfloat16, tag="in", name=f"in_sb{n}")
                iq.dma_start(out=it[:], in_=inT[n * P : (n + 1) * P, :])
                mt = mpool.tile([P, T], mybir.dt.bfloat16, tag="m", name=f"m_sb{n}")
                mq.dma_start(out=mt[:], in_=mmat[n * P : (n + 1) * P, :])
                in_tiles.append(it)
                m_tiles.append(mt)
            # c is first needed by the pass-A evacuation (~43us in); issuing
            # it after the m/in streams keeps it from delaying any m tile.
            nc.sync.dma_start(out=c_sb[:], in_=cvec[:])

            # Two passes of two row-tiles each: 2 PSUM strips of [128, 2048]
            # (4 banks each) per pass fills all 8 banks; the n-loop touches
            # each streamed m tile once per pass so the first pass overlaps
            # the M DMA.
            for half in range(NBT // 2):
                # One 1-bank PSUM tile per (row-tile, k-block) unit: Tile
                # serializes DVE reads vs PE writes at tile granularity, so
                # separate tiles let each bank's evacuation start as soon as
                # its own accumulation group closes.
                ps = [
                    [
                        pspool.tile(
                            [P, KB], mybir.dt.float32, tag="ps", name=f"ps{half}_{bi}_{kb}"
                        )
                        for kb in range(NKB)
                    ]
                    for bi in range(2)
                ]
                if half == 0:
                    # Dummy matmuls on memset data while the first DMAs
                    # land, so the HAM clock gate opens to 8/8 before the
                    # real matmuls. Sized to end just past m0's arrival
                    # (~10.8us) -- more would delay the real stream. Each is
                    # its own start/stop group; the real n==0 matmul
                    # re-clears the bank.
                    for i in range(5):
                        nc.tensor.matmul(ps[0][0][:], wt[:, :P], wt[:])
                def mm(bi, n):
                    bt = half * 2 + bi
                    lhsT = in_tiles[n][:, bt * P : (bt + 1) * P]
                    for kb in range(NKB):
                        nc.tensor.matmul(
                            ps[bi][kb][:],
                            lhsT,
                            m_tiles[n][:, kb * KB : (kb + 1) * KB],
                            start=(n == 0),
                            stop=(n == NJ - 1),
                        )

                # Interleave b-tiles over most of the contraction (matches
                # the M DMA arrival order), but run the last `split`
                # iterations per-b-tile so strip 0's accumulation closes
                # early: its serial DVE evacuation then hides under strip
                # 1's remaining matmuls instead of stalling what follows
                # (pass transition, kernel tail). Pass A keeps split small
                # (m14/m15 DMA arrival is tight); pass B is SBUF-resident.
                split = 4 if half == 0 else 6
                for n in range(NJ - split):
                    for bi in range(2):
                        mm(bi, n)
                for bi in range(2):
                    if half == 1 and bi == 1:
                        # Very last strip: kb-major so each PSUM bank's
                        # accumulation group closes progressively early and
                        # the DVE streams through all 8 tail adds without
                        # idling; only bank 3's add+store trail the last
                        # matmul.
                        bt = half * 2 + bi
                        for kb in range(NKB):
                            for n in range(NJ - split, NJ):
                                nc.tensor.matmul(
                                    ps[bi][kb][:],
                                    in_tiles[n][:, bt * P : (bt + 1) * P],
                                    m_tiles[n][:, kb * KB : (kb + 1) * KB],
                                    start=False,
                                    stop=(n == NJ - 1),
                                )
                    else:
                        for n in range(NJ - split, NJ):
                            mm(bi, n)
                for bi in range(2):
                    bt = half * 2 + bi
                    stq = nc.sync if bi == 0 else nc.scalar
                    ot = opool.tile([P, T], mybir.dt.float32, tag="o", name=f"o_sb{bt}")
                    for kb in range(NKB):
                        ksl = slice(kb * KB, (kb + 1) * KB)
                        nc.vector.tensor_add(
                            out=ot[:, ksl], in0=ps[bi][kb][:], in1=c_sb[:, ksl]
                        )
                        stq.dma_start(
                            out=out[bt * P : (bt + 1) * P, ksl], in_=ot[:, ksl]
                        )

    nc.compile()
    return nc


def _build_M_c(params, bias):
    """M [t, t], c [t] (float64) such that out = inputs @ M + c."""
    t = params.shape[0]
    p_rev = params[::-1].astype(np.float64)
    M = np.empty((t, t), np.float64)
    col = p_rev.copy()
    M[:, 0] = col
    for k in range(1, t):
        last = col[-1]
        shifted = np.empty_like(col)
        shifted[0] = 0.0
        shifted[1:] = col[:-1]
        col = shifted + last * p_rev
        M[:, k] = col
    b = np.float64(bias[0])
    u = np.zeros(t, np.float64)
    c = np.empty(t, np.float64)
    for k in range(t):
        nv = u @ p_rev + b
        c[k] = nv
        u = np.roll(u, -1)
        u[-1] = nv
    return M, c


def _make_in_maps(inputs, params, bias):
    M, c = _build_M_c(params, bias)
    m_bf = M.astype(np.float32).astype(ml_dtypes.bfloat16)
    c128 = np.ascontiguousarray(
        np.broadcast_to(c.astype(np.float32)[None, :], (P, T))
    )
    in_bf = inputs.astype(ml_dtypes.bfloat16)
    in_maps = []
    for s in range(NCORES):
        shard = np.ascontiguousarray(in_bf[s * BS : (s + 1) * BS, :].T)
        in_maps.append({"art_inT": shard, "art_m": m_bf, "art_c": c128})
    return in_maps


def run(inputs, params, bias, **spmd_kwargs):
    """Build in_maps, run the SPMD kernel, return (output, BassKernelResults)."""
    from concourse.bass_utils import run_bass_kernel_spmd

    if "nc" not in _cache:
        _cache["nc"] = _build_and_compile()
    nc = _cache["nc"]

    inputs = np.ascontiguousarray(np.asarray(inputs, dtype=np.float32))
    params = np.asarray(params, dtype=np.float32)
    bias = np.asarray(bias, dtype=np.float32)
    assert inputs.shape == (B, T), inputs.shape
    assert params.shape == (T,), params.shape
    in_maps = _make_in_maps(inputs, params, bias)
    res = run_bass_kernel_spmd(nc, in_maps, core_ids=list(range(NCORES)), **spmd_kwargs)
    out = np.concatenate([r["art_out"] for r in res.results], axis=0)
    return out, res


def kernel(inputs, params, bias):
    out, _ = run(inputs, params, bias)
    return out
